# revision 9
# baseline (speedup 1.0000x reference)
"""DiffusionAttentionPairBias kernel for Trainium2 (8 NeuronCores, SPMD).

Problem (B=1, N=1024, D_A=768, D_S=384, D_Z=128, H=16, DH=48):
  q_in = sigmoid(LN(s) @ gw_ad + gb_ad) * LN(a) + LN(s) @ bw_ad
  q,k,v,g = projections of q_in;  bias = (LN(z)*zn_g + zn_b) @ zp_w
  attn = softmax(q k^T / sqrt(DH) + bias);  out = sigmoid(g) * (attn v)
  y = sigmoid(s @ sg_w + sg_b) * (out @ ow)

Sharding: pure data-parallel on the query axis. Core c owns query rows
[128c, 128c+128): it receives the full a/s (to build k/v for all keys),
its own 128-row slices a_q/s_q/z_q, and computes its 128 output rows.
No collectives; host concatenates.

Key structural choices (per core):
 - z (64MB) is cast f32->bf16 during the HBM load (SWDGE cast-DMA), then
   per-kp [128,128] tiles are xbar-transposed (bf16 HW DMA transpose) to
   feed the ch-contraction projection on the tensor engine.
 - LN over z's 128-ch axis is folded into the projection:
     bias = rstd * (P0 - mean * colsum(W)),  W = zn_g * zp_w
   P0 and the per-(q,k) row-sum come from one 17-column augmented matmul
   [W | 1]; the row sum-of-squares comes from ACT Square+accum_out.
   zn_b's contribution is constant along the softmax axis and cancels.
 - Scores accumulate in PSUM: QK^T matmul + identity-matmul bias fold.
 - Softmax: exact row max (negated reduce_max as the Exp bias), exp on
   ACT (psum->sbuf bf16), sum/reciprocal folded into the AV epilogue.
"""

import math
import os

import numpy as np

import concourse.bass as bass
import concourse.bacc as bacc
import concourse.mybir as mybir
import concourse.tile as tile
from concourse.masks import make_identity

F32 = mybir.dt.float32
BF16 = mybir.dt.bfloat16
AF = mybir.ActivationFunctionType
ALU = mybir.AluOpType
AX = mybir.AxisListType

N = 1024
DA = 768
DS = 384
DZ = 128
H = 16
DH = 48
HD = 768
QP = 128          # query rows per core
NCORES = 8
SCALE = 1.0 / math.sqrt(DH)
EPS = 1e-5
KJ = 16           # kp rows per z macro-tile
NJ = N // KJ      # 64 z macro-tiles


def _bcast(ap, dim, n):
    """Return a copy of `ap` whose `dim`-th AP dim is replaced by [0, n]."""
    dims = [list(d) for d in ap.ap]
    dims[dim] = [0, n]
    return bass.AP(tensor=ap.tensor, offset=ap.offset, ap=dims)


def _bcast_ins(ap, dim, n):
    """Return a copy of `ap` with a broadcast dim [0, n] inserted at `dim`."""
    dims = [list(d) for d in ap.ap]
    dims.insert(dim, [0, n])
    return bass.AP(tensor=ap.tensor, offset=ap.offset, ap=dims)


def build_program():
    nc = bacc.Bacc("TRN2", target_bir_lowering=False, debug=False)

    dram = {}

    def din(name, shape):
        dram[name] = nc.dram_tensor(name, shape, F32, kind="ExternalInput")
        return dram[name]

    a_d = din("a", [N, DA])
    s_d = din("s", [N, DS])
    aq_d = din("a_q", [QP, DA])
    sq_d = din("s_q", [QP, DS])
    z_d = din("z_q", [QP, N, DZ])
    gwad_d = din("adaln_gw", [DS, DA])
    bwad_d = din("adaln_bw", [DS, DA])
    gbad_d = din("adaln_gb", [DA])
    qw_d = din("qw", [DA, HD])
    qb_d = din("qb", [HD])
    kw_d = din("kw", [DA, HD])
    vw_d = din("vw", [DA, HD])
    gw_d = din("gw", [DA, HD])
    ow_d = din("ow", [HD, DA])
    zng_d = din("zn_g", [DZ])
    zpw_d = din("zp_w", [DZ, H])
    sgw_d = din("sg_w", [DS, DA])
    sgb_d = din("sg_b", [DA])
    out_d = nc.dram_tensor("out", [QP, DA], F32, kind="ExternalOutput")

    with tile.TileContext(nc) as tc:
        with (
            tc.tile_pool(name="const", bufs=1) as cp,
            tc.tile_pool(name="persist", bufs=1) as pp,
            tc.tile_pool(name="wpool", bufs=2) as wp,
            tc.tile_pool(name="act", bufs=2) as ap_,
            tc.tile_pool(name="zpipe", bufs=2) as zp,
            tc.tile_pool(name="ztr", bufs=4) as ztp,
            tc.tile_pool(name="attnp", bufs=2) as atp,
            tc.tile_pool(name="ps_z", bufs=2, space="PSUM") as ps_z,
            tc.tile_pool(name="ps_s", bufs=2, space="PSUM") as ps_s,
            tc.tile_pool(name="ps_m", bufs=1, space="PSUM") as ps_m,
        ):
            # ---------------- constants / small prep ----------------
            ones_r = cp.tile([1, 128], BF16)
            nc.vector.memset(ones_r, 1.0)
            epsA = cp.tile([128, 1], F32)
            nc.vector.memset(epsA, EPS)
            ident = cp.tile([128, 128], BF16)
            make_identity(nc, ident)

            zng_c = cp.tile([DZ, 1], F32)
            nc.sync.dma_start(out=zng_c, in_=zng_d[:].rearrange("(p o) -> p o", o=1))
            zpw_f = cp.tile([DZ, H], F32)
            nc.sync.dma_start(out=zpw_f, in_=zpw_d[:])
            # W_aug = [zn_g*zp_w | ones]  (bf16, lhs of the z projection)
            waug = cp.tile([DZ, H + 1], BF16)
            nc.vector.tensor_scalar_mul(waug[:, 0:H], in0=zpw_f[:], scalar1=zng_c[:])
            nc.vector.memset(waug[:, H : H + 1], 1.0)

            # colsum(W)/128 broadcast to all 128 partitions via rank-1 matmul
            ones_col = cp.tile([DZ, 1], BF16)
            nc.vector.memset(ones_col, 1.0)
            ps_cs = ps_m.tile([1, H + 1], F32, tag="misc")
            nc.tensor.matmul(ps_cs, ones_col[:], waug[:], start=True, stop=True)
            cs_row = cp.tile([1, H], BF16)
            nc.scalar.activation(cs_row, ps_cs[0:1, 0:H], AF.Copy, scale=1.0 / DZ)
            ps_csb = ps_m.tile([128, H], F32, tag="misc")
            nc.tensor.matmul(ps_csb, ones_r[:], cs_row[:], start=True, stop=True)
            csD = cp.tile([128, H], F32)
            nc.vector.tensor_copy(csD, ps_csb[:])

            # per-feature bias rows (bf16, added via K=1 rank-1 matmuls)
            gbad_r = cp.tile([1, DA], BF16)
            nc.gpsimd.dma_start(out=gbad_r, in_=gbad_d[:].rearrange("(o n) -> o n", o=1))
            qb_r = cp.tile([1, HD], BF16)
            nc.gpsimd.dma_start(out=qb_r, in_=qb_d[:].rearrange("(o n) -> o n", o=1))
            sgb_r = cp.tile([1, DA], BF16)
            nc.gpsimd.dma_start(out=sgb_r, in_=sgb_d[:].rearrange("(o n) -> o n", o=1))

            # ---------------- persistent activations ----------------
            s_lnT = pp.tile([128, 3, N], BF16)        # LN(s)^T, all positions
            q_inT = pp.tile([128, 6, N], BF16)        # q_in^T, all positions
            qi_qT = pp.tile([128, 6, QP], BF16)       # q_in^T, this core's rows
            kT = pp.tile([128, 8, N], BF16)           # K^T head-pairs at part 0/64
            qT = pp.tile([128, 8, QP], BF16)          # (Q+qb)*SCALE^T head-pairs
            v_sb = pp.tile([128, 8, HD], BF16)        # V natural, all positions
            sig_g = pp.tile([128, HD], F32)
            sig_o = pp.tile([128, DA], F32)
            out_nat = pp.tile([128, HD], BF16)
            bias_h = [
                pp.tile([128, 512, H], BF16, tag=f"bias{i}", name=f"bias{i}")
                for i in range(2)
            ]

            def ln_tile(src_ap, cols, out_bf):
                """LayerNorm rows of [128, cols] -> bf16 tile (no affine)."""
                xt = ap_.tile([128, cols], F32, tag="lnin")
                nc.sync.dma_start(out=xt, in_=src_ap)
                st6 = ap_.tile([128, 2, 6], F32, tag="lnst")
                half = cols // 2
                nc.vector.bn_stats(out=st6[:, 0, :], in_=xt[:, 0:half])
                nc.vector.bn_stats(out=st6[:, 1, :], in_=xt[:, half:cols])
                mv = ap_.tile([128, 2], F32, tag="lnmv")
                nc.vector.bn_aggr(out=mv, in_=st6[:, :, :])
                sd = ap_.tile([128, 1], F32, tag="lnsd")
                nc.scalar.activation(sd, mv[:, 1:2], AF.Sqrt, bias=epsA[:])
                rs = ap_.tile([128, 1], F32, tag="lnrs")
                nc.vector.reciprocal(rs, sd[:])
                nc.vector.tensor_scalar(
                    out=out_bf,
                    in0=xt[:],
                    scalar1=mv[:, 0:1],
                    scalar2=rs[:],
                    op0=ALU.subtract,
                    op1=ALU.mult,
                )

            # ---- LN(s) for all 8 tiles -> s_lnT via xbar transposes ----
            for t in range(8):
                s_ln = ap_.tile([128, DS], BF16, tag="sln")
                ln_tile(s_d[t * 128 : (t + 1) * 128, :], DS, s_ln[:])
                for kt in range(3):
                    nc.sync.dma_start(
                        out=s_lnT[:, kt, t * 128 : (t + 1) * 128],
                        in_=s_ln[:, kt * 128 : (kt + 1) * 128],
                        transpose=True,
                    )

            # ---- adaln -> q_in (all positions) -> q_inT ----
            gwad_s = wp.tile([128, 3, DA], BF16, tag="w9")
            nc.gpsimd.dma_start(out=gwad_s, in_=gwad_d[:].rearrange("(t p) n -> p t n", p=128))
            bwad_s = wp.tile([128, 3, DA], BF16, tag="w9")
            nc.gpsimd.dma_start(out=bwad_s, in_=bwad_d[:].rearrange("(t p) n -> p t n", p=128))

            chunks = [(0, 512), (512, 256)]

            def adaln_qin(lnT_ap, a_src, pos0, out_T, outT_col0):
                """q_in rows for 128 positions; lnT_ap[kt] -> [128,128] lhsT."""
                a_ln = ap_.tile([128, DA], BF16, tag="aln")
                ln_tile(a_src, DA, a_ln[:])
                psG = ps_m.tile([128, DA], F32, tag="misc")
                for c0, cn in chunks:
                    for kt in range(3):
                        nc.tensor.matmul(
                            psG[:, c0 : c0 + cn],
                            lnT_ap(kt),
                            gwad_s[:, kt, c0 : c0 + cn],
                            start=(kt == 0),
                            stop=False,
                        )
                    nc.tensor.matmul(
                        psG[:, c0 : c0 + cn],
                        ones_r[:],
                        gbad_r[:, c0 : c0 + cn],
                        start=False,
                        stop=True,
                    )
                sgG = ap_.tile([128, DA], F32, tag="sgG")
                nc.scalar.activation(sgG, psG[:], AF.Sigmoid)
                psB = ps_m.tile([128, DA], F32, tag="misc")
                for c0, cn in chunks:
                    for kt in range(3):
                        nc.tensor.matmul(
                            psB[:, c0 : c0 + cn],
                            lnT_ap(kt),
                            bwad_s[:, kt, c0 : c0 + cn],
                            start=(kt == 0),
                            stop=(kt == 2),
                        )
                tmp = ap_.tile([128, DA], F32, tag="qtmp")
                nc.vector.tensor_mul(tmp, sgG[:], a_ln[:])
                q_in = ap_.tile([128, DA], BF16, tag="qin")
                nc.vector.tensor_add(q_in, tmp[:], psB[:])
                for kt in range(6):
                    nc.sync.dma_start(
                        out=out_T[:, kt, outT_col0 : outT_col0 + 128],
                        in_=q_in[:, kt * 128 : (kt + 1) * 128],
                        transpose=True,
                    )

            for t in range(8):
                adaln_qin(
                    lambda kt, t=t: s_lnT[:, kt, t * 128 : (t + 1) * 128],
                    a_d[t * 128 : (t + 1) * 128, :],
                    t * 128,
                    q_inT,
                    t * 128,
                )

            # q-row version (recomputed from a_q/s_q so the program is SPMD)
            sq_ln = pp.tile([128, DS], BF16)
            ln_tile(sq_d[:], DS, sq_ln[:])
            sq_lnT = pp.tile([128, 3, QP], BF16)
            for kt in range(3):
                nc.sync.dma_start(
                    out=sq_lnT[:, kt, :],
                    in_=sq_ln[:, kt * 128 : (kt + 1) * 128],
                    transpose=True,
                )
            adaln_qin(lambda kt: sq_lnT[:, kt, :], aq_d[:], 0, qi_qT, 0)

            # ---- K^T head-pairs and V natural ----
            kw_s = wp.tile([128, 6, HD], BF16, tag="w9")
            nc.gpsimd.dma_start(out=kw_s, in_=kw_d[:].rearrange("(t p) n -> p t n", p=128))
            for p in range(8):
                for half in range(2):
                    c0 = half * 512
                    psK = ps_m.tile([128, 512], F32, tag="misc")
                    for sub in range(2):
                        h = 2 * p + sub
                        off = 64 * sub
                        for kt in range(6):
                            nc.tensor.matmul(
                                psK[off : off + 48, :],
                                kw_s[:, kt, 48 * h : 48 * h + 48],
                                q_inT[:, kt, c0 : c0 + 512],
                                start=(kt == 0),
                                stop=(kt == 5),
                            )
                        nc.vector.tensor_copy(
                            kT[off : off + 48, p, c0 : c0 + 512], psK[off : off + 48, :]
                        )

            vw_s = wp.tile([128, 6, HD], BF16, tag="w9")
            nc.gpsimd.dma_start(out=vw_s, in_=vw_d[:].rearrange("(t p) n -> p t n", p=128))
            for t in range(8):
                for c0, cn in chunks:
                    psV = ps_m.tile([128, 512], F32, tag="misc")
                    for kt in range(6):
                        nc.tensor.matmul(
                            psV[:, 0:cn],
                            q_inT[:, kt, t * 128 : (t + 1) * 128],
                            vw_s[:, kt, c0 : c0 + cn],
                            start=(kt == 0),
                            stop=(kt == 5),
                        )
                    nc.vector.tensor_copy(v_sb[:, t, c0 : c0 + cn], psV[:, 0:cn])

            # ---- Q^T head-pairs (scaled, biased) ----
            qw_s = wp.tile([128, 6, HD], BF16, tag="w9")
            nc.gpsimd.dma_start(out=qw_s, in_=qw_d[:].rearrange("(t p) n -> p t n", p=128))
            for p in range(8):
                psQ = ps_m.tile([128, QP], F32, tag="misc")
                for sub in range(2):
                    h = 2 * p + sub
                    off = 64 * sub
                    for kt in range(6):
                        nc.tensor.matmul(
                            psQ[off : off + 48, :],
                            qw_s[:, kt, 48 * h : 48 * h + 48],
                            qi_qT[:, kt, :],
                            start=(kt == 0),
                            stop=False,
                        )
                    nc.tensor.matmul(
                        psQ[off : off + 48, :],
                        qb_r[:, 48 * h : 48 * h + 48],
                        ones_r[:],
                        start=False,
                        stop=True,
                    )
                    nc.scalar.activation(
                        qT[off : off + 48, p, :], psQ[off : off + 48, :], AF.Copy, scale=SCALE
                    )

            # ---- G gate ----
            gw_s = wp.tile([128, 6, HD], BF16, tag="w9")
            nc.gpsimd.dma_start(out=gw_s, in_=gw_d[:].rearrange("(t p) n -> p t n", p=128))
            for c0, cn in chunks:
                psg = ps_m.tile([128, 512], F32, tag="misc")
                for kt in range(6):
                    nc.tensor.matmul(
                        psg[:, 0:cn],
                        qi_qT[:, kt, :],
                        gw_s[:, kt, c0 : c0 + cn],
                        start=(kt == 0),
                        stop=(kt == 5),
                    )
                nc.scalar.activation(sig_g[:, c0 : c0 + cn], psg[:, 0:cn], AF.Sigmoid)

            # ---- output gate from raw s_q ----
            sgw_s = wp.tile([128, 3, DA], BF16, tag="w9")
            nc.gpsimd.dma_start(out=sgw_s, in_=sgw_d[:].rearrange("(t p) n -> p t n", p=128))
            sq_bf = ap_.tile([128, DS], BF16, tag="sqbf")
            nc.gpsimd.dma_start(out=sq_bf, in_=sq_d[:])
            sqT = pp.tile([128, 3, QP], BF16)
            for kt in range(3):
                nc.sync.dma_start(
                    out=sqT[:, kt, :],
                    in_=sq_bf[:, kt * 128 : (kt + 1) * 128],
                    transpose=True,
                )
            for c0, cn in chunks:
                pso = ps_m.tile([128, 512], F32, tag="misc")
                for kt in range(3):
                    nc.tensor.matmul(
                        pso[:, 0:cn],
                        sqT[:, kt, :],
                        sgw_s[:, kt, c0 : c0 + cn],
                        start=(kt == 0),
                        stop=False,
                    )
                nc.tensor.matmul(
                    pso[:, 0:cn], ones_r[:], sgb_r[:, c0 : c0 + cn], start=False, stop=True
                )
                nc.scalar.activation(sig_o[:, c0 : c0 + cn], pso[:, 0:cn], AF.Sigmoid)

            # ---------------- z pipeline: pair bias ----------------
            for j in range(NJ):
                k0 = j * KJ
                zt = zp.tile([128, KJ, DZ], BF16, tag="zt")
                nc.gpsimd.dma_start(out=zt, in_=z_d[:, k0 : k0 + KJ, :])
                sqt = zp.tile([128, KJ, 1], F32, tag="sqt")
                scr = zp.tile([128, DZ], BF16, tag="scr")
                p0 = ps_z.tile([128, KJ, H + 1], F32, tag="p0")
                for i in range(KJ):
                    nc.scalar.activation(
                        scr, zt[:, i, :], AF.Square, accum_out=sqt[:, i, :]
                    )
                    zT = ztp.tile([128, 128], BF16, tag="zT")
                    nc.sync.dma_start(out=zT, in_=zt[:, i, :], transpose=True)
                    nc.tensor.matmul(p0[:, i, :], zT[:], waug[:], start=True, stop=True)
                # rstd = 1/sqrt((SQ - S1^2/128)/128 + eps)
                s1c = zp.tile([128, KJ, 1], F32, tag="s1c")
                nc.vector.tensor_copy(s1c, p0[:, :, H : H + 1])
                x1 = zp.tile([128, KJ, 1], F32, tag="x1")
                nc.vector.tensor_mul(x1, s1c[:], s1c[:])
                x2 = zp.tile([128, KJ, 1], F32, tag="x2")
                nc.vector.scalar_tensor_tensor(
                    out=x2,
                    in0=x1[:],
                    scalar=-1.0 / DZ,
                    in1=sqt[:],
                    op0=ALU.mult,
                    op1=ALU.add,
                )
                sd = zp.tile([128, KJ, 1], F32, tag="zsd")
                nc.scalar.activation(sd, x2[:], AF.Sqrt, scale=1.0 / DZ, bias=epsA[:])
                rstd = zp.tile([128, KJ, 1], F32, tag="zrs")
                nc.vector.reciprocal(rstd, sd[:])
                # bias = rstd * (P0 - S1 * colsum(W)/128)
                t1 = zp.tile([128, KJ, H], F32, tag="t1")
                nc.vector.tensor_mul(
                    t1, _bcast(p0[:, :, H : H + 1], 2, H), _bcast_ins(csD[:], 1, KJ)
                )
                t2 = zp.tile([128, KJ, H], F32, tag="t2")
                nc.vector.tensor_sub(t2, p0[:, :, 0:H], t1[:])
                half = k0 // 512
                nc.vector.tensor_mul(
                    bias_h[half][:, k0 - half * 512 : k0 - half * 512 + KJ, :],
                    t2[:],
                    _bcast(rstd[:], 2, H),
                )

            # ---------------- attention ----------------
            for h in range(H):
                po = 64 * (h % 2)
                pr = h // 2
                sc = ps_s.tile([128, N], F32, tag="sc")
                for half in range(2):
                    c0 = half * 512
                    nc.tensor.matmul(
                        sc[:, c0 : c0 + 512],
                        qT[po : po + 48, pr, :],
                        kT[po : po + 48, pr, c0 : c0 + 512],
                        start=True,
                        stop=False,
                    )
                    nc.tensor.matmul(
                        sc[:, c0 : c0 + 512],
                        ident[:],
                        bias_h[half][:, :, h],
                        start=False,
                        stop=True,
                    )
                nmx = atp.tile([128, 1], F32, tag="nmx")
                nc.vector.reduce_max(out=nmx, in_=sc[:], axis=AX.X, negate=True)
                attn = atp.tile([128, N], BF16, tag="attn")
                nc.scalar.activation(attn, sc[:], AF.Exp, bias=nmx[:])
                den = atp.tile([128, 1], F32, tag="den")
                nc.vector.reduce_sum(out=den, in_=attn[:], axis=AX.X)
                rden = atp.tile([128, 1], F32, tag="rden")
                nc.vector.reciprocal(rden, den[:])
                attnT = atp.tile([128, 8, 128], BF16, tag="attnT")
                for kt in range(8):
                    nc.sync.dma_start(
                        out=attnT[:, kt, :],
                        in_=attn[:, kt * 128 : (kt + 1) * 128],
                        transpose=True,
                    )
                psA = ps_m.tile([128, DH], F32, tag="misc")
                for kt in range(8):
                    nc.tensor.matmul(
                        psA,
                        attnT[:, kt, :],
                        v_sb[:, kt, DH * h : DH * h + DH],
                        start=(kt == 0),
                        stop=(kt == 7),
                    )
                nc.vector.scalar_tensor_tensor(
                    out=out_nat[:, DH * h : DH * h + DH],
                    in0=psA[:],
                    scalar=rden[:, 0:1],
                    in1=sig_g[:, DH * h : DH * h + DH],
                    op0=ALU.mult,
                    op1=ALU.mult,
                )

            # ---------------- output projection ----------------
            outT = pp.tile([128, 6, QP], BF16)
            for kt in range(6):
                nc.sync.dma_start(
                    out=outT[:, kt, :],
                    in_=out_nat[:, kt * 128 : (kt + 1) * 128],
                    transpose=True,
                )
            ow_s = pp.tile([128, 6, DA], BF16)
            nc.gpsimd.dma_start(out=ow_s, in_=ow_d[:].rearrange("(t p) n -> p t n", p=128))
            fin = pp.tile([128, DA], F32)
            for c0, cn in chunks:
                psF = ps_m.tile([128, 512], F32, tag="misc")
                for kt in range(6):
                    nc.tensor.matmul(
                        psF[:, 0:cn],
                        outT[:, kt, :],
                        ow_s[:, kt, c0 : c0 + cn],
                        start=(kt == 0),
                        stop=(kt == 5),
                    )
                nc.vector.tensor_mul(fin[:, c0 : c0 + cn], psF[:, 0:cn], sig_o[:, c0 : c0 + cn])
            nc.sync.dma_start(out=out_d[:], in_=fin[:])

    nc.compile()
    return nc


_CACHE = {}


def _get_program():
    if "nc" not in _CACHE:
        _CACHE["nc"] = build_program()
    return _CACHE["nc"]


def make_in_maps(inputs):
    """Shard full inputs into 8 per-core input maps."""
    f = lambda k: np.ascontiguousarray(np.asarray(inputs[k], dtype=np.float32))
    a = f("a")[0]
    s = f("s")[0]
    z = f("z")[0]
    shared = {
        "a": a,
        "s": s,
        "adaln_gw": f("adaln_gw"),
        "adaln_bw": f("adaln_bw"),
        "adaln_gb": f("adaln_gb"),
        "qw": f("qw"),
        "qb": f("qb"),
        "kw": f("kw"),
        "vw": f("vw"),
        "gw": f("gw"),
        "ow": f("ow"),
        "zn_g": f("zn_g"),
        "zp_w": f("zp_w"),
        "sg_w": f("sg_w"),
        "sg_b": f("sg_b"),
    }
    in_maps = []
    for c in range(NCORES):
        sl = slice(c * QP, (c + 1) * QP)
        m = dict(shared)
        m["a_q"] = np.ascontiguousarray(a[sl])
        m["s_q"] = np.ascontiguousarray(s[sl])
        m["z_q"] = np.ascontiguousarray(z[sl])
        in_maps.append(m)
    return in_maps


def kernel(**inputs) -> np.ndarray:
    from concourse.bass_utils import run_bass_kernel_spmd

    nc = _get_program()
    in_maps = make_in_maps(inputs)
    trace = bool(int(os.environ.get("KERNEL_TRACE", "0")))
    try:
        res = run_bass_kernel_spmd(
            nc, in_maps, core_ids=list(range(NCORES)), trace=trace
        )
    except ModuleNotFoundError:
        res = run_bass_kernel_spmd(
            nc, in_maps, core_ids=list(range(NCORES)), trace=False
        )
    _CACHE["last_results"] = res
    out = np.concatenate([res.results[c]["out"] for c in range(NCORES)], axis=0)
    return out[None].astype(np.float32)
